# revision 5
# baseline (speedup 1.0000x reference)
"""CrossCosineEmbeddingLoss kernel for 8 trn2 NeuronCores (v5).

loss = mean over all (i,j) of: 1 - cos(x_i, y_j) if i==j else relu(cos(x_i, y_j))

Identity:  total = sum_ij relu(sim_ij) + sum_i (1 - sim_ii - relu(sim_ii))

Sharding (2x4 grid): core c = (bi, bj), bi = c // 2, bj = c % 2.
  x rows [2048*bi, 2048*(bi+1)) x y rows [4096*bj, 4096*(bj+1)).
Each core computes sum_ij relu(x_hat_i . y_j) / ||y_j|| over its block.
Diag correction only used from cores whose x block lies in their y range.

Per-core pipeline:
  - y half: SWDGE cast-DMA fp32->bf16 (2 chunks); GpSimd squares + DVE
    segmented reduce for row norms; PE transpose (bf16) + DVE copy -> yT
  - x shard: GpSimd square + DVE reduce, rsqrt, DVE scale-cast to bf16
    x_hat, PE transpose -> x_hatT bf16
  - main: 32 j-tiles, each [128j, 2048i] fp32 PSUM (4 banks): 4 bf16
    matmuls (N=512); reduce:
      ACT tiles (even t + 31): activation(Relu, scale=rny_col, accum_out)
      DVE tiles (odd t):       tensor_scalar(max 0, add, accum_out) then
                               R column post-scaled by rny
  - diag: GpSimd products + DVE reduces, small fp32 ops
Host combines [128,2] partials; diag col used only from owner cores.
"""

import numpy as np

import concourse.bacc as bacc
import concourse.bass as bass
import concourse.tile as tile
from concourse import mybir
from concourse.bass_utils import run_bass_kernel_spmd
from concourse.masks import make_identity

N, D = 8192, 128
NCORES = 8
XI = 2048            # x rows per core
YJ = 4096            # y rows per core
TXI = XI // 128      # 16 x tiles
TYJ = YJ // 128      # 32 y j-tiles
YCH = 2              # y cast-DMA chunks
YCT = TYJ // YCH     # 16 j-tiles per chunk

f32 = mybir.dt.float32
bf16 = mybir.dt.bfloat16
AF = mybir.ActivationFunctionType
ALU = mybir.AluOpType
AX = mybir.AxisListType

# DVE tiles: odd t (15), ACT tiles: even t and t=31 (17)
def _kind(t):
    return "dve" if (t % 2 == 1 and t != 31) else "act"


_CACHE = {}


def _build():
    if "nc" in _CACHE:
        return _CACHE["nc"]
    nc = bacc.Bacc("TRN2", target_bir_lowering=False, debug=False,
                   num_devices=NCORES)
    xs_d = nc.dram_tensor("xs", [XI, D], f32, kind="ExternalInput")
    y_d = nc.dram_tensor("y", [YJ, D], f32, kind="ExternalInput")
    yd_d = nc.dram_tensor("yd", [XI, D], f32, kind="ExternalInput")
    out_d = nc.dram_tensor("out", [128, 2], f32, kind="ExternalOutput")

    with tile.TileContext(nc) as tc:
        with (
            tc.tile_pool(name="singles", bufs=1) as singles,
            tc.tile_pool(name="scr", bufs=2) as scr,
        ):
            ident = singles.tile([128, 128], bf16)
            make_identity(nc, ident[:])

            ynat = singles.tile([128, TYJ, 128], bf16)   # row j: 2048g+16p+t
            yT = singles.tile([128, TYJ, 128], bf16)     # [d, t, j-col]
            xnat = singles.tile([128, TXI, 128], f32)    # row i = 16p + t
            xhat = singles.tile([128, TXI, 128], bf16)
            xhatT = singles.tile([128, TXI * 128], bf16)
            ydn = singles.tile([128, TXI, 128], f32)

            ny2 = singles.tile([128, TYJ], bf16)
            t2y = singles.tile([128, TYJ], f32)
            rny = singles.tile([128, TYJ], f32)
            nx2 = singles.tile([128, TXI], bf16)
            nyd2 = singles.tile([128, TXI], bf16)
            t1x = singles.tile([128, TXI], f32)
            rnx = singles.tile([128, TXI], f32)
            rnyd = singles.tile([128, TXI], f32)
            d2 = singles.tile([128, TXI], bf16)
            sim_d = singles.tile([128, TXI], f32)
            relu_d = singles.tile([128, TXI], f32)
            R = singles.tile([128, 64], f32)
            outsb = singles.tile([128, 2], f32)

            # ---- input DMAs
            for g in range(YCH):
                rows = slice(2048 * g, 2048 * (g + 1))
                nc.gpsimd.dma_start(
                    out=ynat[:, YCT * g:YCT * (g + 1), :],
                    in_=y_d[rows].rearrange("(p t) d -> p t d", t=YCT))
            nc.sync.dma_start(
                out=xnat[:], in_=xs_d[:].rearrange("(p t) d -> p t d", t=TXI))
            nc.sync.dma_start(
                out=ydn[:], in_=yd_d[:].rearrange("(p t) d -> p t d", t=TXI))

            with nc.allow_low_precision("norm sums in bf16 are plenty"):
                # ---- x norms: GpSimd square, DVE segmented reduce
                xsq = scr.tile([128, TXI, 128], bf16, tag="sq", name="xsq")
                nc.gpsimd.tensor_mul(
                    xsq[:].rearrange("p a b -> p (a b)"),
                    xnat[:].rearrange("p a b -> p (a b)"),
                    xnat[:].rearrange("p a b -> p (a b)"))
                nc.vector.tensor_reduce(out=nx2[:], in_=xsq[:], axis=AX.X,
                                        op=ALU.add)
                nc.vector.reciprocal(t1x[:], nx2[:])
                nc.scalar.sqrt(rnx[:], t1x[:])   # 1/||x_i||
                for t in range(TXI):
                    nc.vector.tensor_scalar(
                        out=xhat[:, t, :], in0=xnat[:, t, :],
                        scalar1=rnx[:, t:t + 1], scalar2=None,
                        op0=ALU.mult)

                # ---- y norms: GpSimd squares + DVE reduces (per chunk)
                for g in range(YCH):
                    ysq = scr.tile([128, YCT, 128], bf16, tag="sq",
                                   name="ysq")
                    gs = slice(YCT * g, YCT * (g + 1))
                    nc.gpsimd.tensor_mul(
                        ysq[:].rearrange("p a b -> p (a b)"),
                        ynat[:, gs, :].rearrange("p a b -> p (a b)"),
                        ynat[:, gs, :].rearrange("p a b -> p (a b)"))
                    nc.vector.tensor_reduce(out=ny2[:, gs], in_=ysq[:],
                                            axis=AX.X, op=ALU.add)
                nc.vector.reciprocal(t2y[:], ny2[:])
                nc.scalar.sqrt(rny[:], t2y[:])   # 1/||y_j||

            # ---- transposes on PE (bf16) + DVE copies to SBUF
            with tc.tile_pool(name="tpsum", bufs=2, space="PSUM") as tpsum:
                ptx = tpsum.tile([128, 2048], bf16, tag="tp")
                for t in range(TXI):
                    nc.tensor.transpose(ptx[:, 128 * t:128 * (t + 1)],
                                        xhat[:, t, :], ident[:])
                nc.vector.tensor_copy(out=xhatT[:], in_=ptx[:])
                for g in range(YCH):
                    pty = tpsum.tile([128, 2048], bf16, tag="tp")
                    for k in range(YCT):
                        t = YCT * g + k
                        nc.tensor.transpose(pty[:, 128 * k:128 * (k + 1)],
                                            ynat[:, t, :], ident[:])
                    nc.vector.tensor_copy(
                        out=yT[:, YCT * g:YCT * (g + 1), :]
                        .rearrange("p a b -> p (a b)"),
                        in_=pty[:])

            # ---- main loop: 32 j-tiles of [128, 2048]
            rcol_of = {}
            rcol = 0
            with (
                tc.tile_pool(name="pA", bufs=1, space="PSUM") as poolA,
                tc.tile_pool(name="pD", bufs=1, space="PSUM") as poolD,
            ):
                for t in range(TYJ):
                    lhsT = yT[:, t, :]
                    pool = poolA if _kind(t) == "act" else poolD
                    ps = pool.tile([128, 2048], f32, tag="ps")
                    for k in range(4):
                        nc.tensor.matmul(ps[:, 512 * k:512 * (k + 1)],
                                         lhsT,
                                         xhatT[:, 512 * k:512 * (k + 1)])
                    if _kind(t) == "act":
                        nc.scalar.activation(
                            ps[:], ps[:], AF.Relu, scale=rny[:, t:t + 1],
                            accum_out=R[:, rcol:rcol + 1])
                    else:
                        nc.vector.tensor_scalar(
                            out=ps[:], in0=ps[:], scalar1=0.0, scalar2=None,
                            op0=ALU.max, op1=ALU.add,
                            accum_out=R[:, rcol:rcol + 1])
                    rcol_of[t] = rcol
                    rcol += 1

            # post-scale DVE R columns by rny (dve tiles are odd t)
            dve_ts = [t for t in range(TYJ) if _kind(t) == "dve"]
            c0, c1 = rcol_of[dve_ts[0]], rcol_of[dve_ts[-1]]
            # odd t are contiguous rcols? pattern act,dve alternating -> rcols
            # for dve tiles are 1,3,5,... stride 2; rny cols are odd stride 2
            nc.vector.tensor_mul(R[:, c0:c1 + 1:2], R[:, c0:c1 + 1:2],
                                 rny[:, 1:TYJ - 1:2])

            # ---- diagonal: sim_ii for local x rows (fp32 via GpSimd/DVE)
            with nc.allow_low_precision("diag sums in bf16 are plenty"):
                prod = scr.tile([128, TXI, 128], bf16, tag="sq", name="prod")
                nc.gpsimd.tensor_mul(
                    prod[:].rearrange("p a b -> p (a b)"),
                    xnat[:].rearrange("p a b -> p (a b)"),
                    ydn[:].rearrange("p a b -> p (a b)"))
                nc.vector.tensor_reduce(out=d2[:], in_=prod[:], axis=AX.X,
                                        op=ALU.add)
                ydsq = scr.tile([128, TXI, 128], bf16, tag="sq", name="ydsq")
                nc.gpsimd.tensor_mul(
                    ydsq[:].rearrange("p a b -> p (a b)"),
                    ydn[:].rearrange("p a b -> p (a b)"),
                    ydn[:].rearrange("p a b -> p (a b)"))
                nc.vector.tensor_reduce(out=nyd2[:], in_=ydsq[:], axis=AX.X,
                                        op=ALU.add)
                nc.vector.reciprocal(t1x[:], nyd2[:])
            nc.scalar.sqrt(rnyd[:], t1x[:])
            # sim_ii = d2 * rnx * rnyd  (d2 used raw x, so rnx applies)
            nc.vector.tensor_mul(sim_d[:], d2[:], rnx[:])
            nc.vector.tensor_mul(sim_d[:], sim_d[:], rnyd[:])
            nc.scalar.activation(relu_d[:], sim_d[:], AF.Relu)
            nc.vector.scalar_tensor_tensor(
                out=scr.tile([128, TXI], f32, tag="dd", name="dd")[:],
                in0=sim_d[:], scalar=1.0, in1=relu_d[:],
                op0=ALU.mult, op1=ALU.add, accum_out=outsb[:, 1:2])

            # ---- final: sum R columns
            nc.vector.tensor_reduce(out=outsb[:, 0:1], in_=R[:, 0:rcol],
                                    axis=AX.X, op=ALU.add)
            nc.sync.dma_start(out=out_d[:], in_=outsb[:])

    nc.compile()
    _CACHE["nc"] = nc
    return nc


# cores whose x block lies inside their y range own the diag correction
_DIAG_OWNER = [1, 0, 1, 0, 0, 1, 0, 1]


def _in_maps(x, y):
    maps = []
    for c in range(NCORES):
        bi, bj = c // 2, c % 2
        xsl = slice(XI * bi, XI * (bi + 1))
        ysl = slice(YJ * bj, YJ * (bj + 1))
        maps.append({"xs": np.ascontiguousarray(x[xsl]),
                     "y": np.ascontiguousarray(y[ysl]),
                     "yd": np.ascontiguousarray(y[xsl])})
    return maps


def _combine(results):
    total = 0.0
    for c in range(NCORES):
        o = results[c]["out"].astype(np.float64)
        total += o[:, 0].sum()
        if _DIAG_OWNER[c]:
            total += XI - o[:, 1].sum()
    return np.float32(total / (float(N) * float(N)))


def _run(x, y, trace=False):
    nc = _build()
    res = run_bass_kernel_spmd(nc, _in_maps(x, y), list(range(NCORES)),
                               trace=trace)
    return _combine(res.results), res


def kernel(x, y):
    x = np.asarray(x, dtype=np.float32)
    y = np.asarray(y, dtype=np.float32)
    loss, _ = _run(x, y, trace=False)
    return loss
